# revision 1
# baseline (speedup 1.0000x reference)
"""AffinityLoss kernel for 8 Trainium2 NeuronCores (Bass/Tile, SPMD).

loss = mean over levels of mean(|softmax_b(G1) - softmax_b(G2)|), where
G[b] = r[b].T @ r[b] is the per-batch Gram matrix over hw pixels and the
softmax runs over the batch axis (b=4).

Strategy
--------
- Shard Gram ROWS across the 8 cores. Since |A1-A2| is symmetric in (i,j),
  only the upper triangle is computed (level 0); row-blocks are assigned to
  cores in a balanced mix {k, 15-k, 16+k, 31-k} so every core gets exactly
  34 level-0 tile positions. Level 1 is small and processed flat.
- One uniform SPMD program; all per-core variation (which rows / which
  j-columns / diagonal masks) is baked into host-prepared input data, so a
  single compiled NEFF runs on all 8 cores.
- Per position (128-row block x 256-col j-tile, both feature sets at once):
    PE:   8 matmuls (fp32) -> PSUM [128, 2, 4, 256] (side, batch, j)
    DVE:  m = max_b G (strided reduce); gsub = G - m  (softmax is invariant
          to per-(i,j) shifts; after this, exp args <= 0 and s in [1,4],
          so there is no overflow/underflow for any input data)
    ACT:  e = exp(gsub) -> bf16
    DVE:  s = sum_b e; rinv = 1/s; A = e*rinv; d = A1 - A2  (bf16, 2x mode)
    ACT:  |d| with accum_out -> per-position per-partition partial sums
- Host gathers the [128, 38] partial-sum tiles from the 8 cores and reduces
  in float64. Diagonal-straddling tiles carry {1, 0.5, 0} masks so that
  2*sum(upper) reproduces the full-matrix sum exactly.
"""

import numpy as np
import ml_dtypes

import concourse.bass as bass
import concourse.mybir as mybir
import concourse.tile as tile
from concourse.bass_utils import run_bass_kernel_spmd

F32 = mybir.dt.float32
BF16 = mybir.dt.bfloat16

B = 4
C0, HW0 = 64, 4096     # level 0: [4, 64, 64, 64]
C1, HW1 = 128, 1024    # level 1: [4, 128, 32, 32]
NCORES = 8
JT = 256               # j-tile width
RB = 128               # row-block height (partition dim)

N_L1 = HW1 // JT                    # 4
N_L0_TRI = 34                       # balanced upper-triangle positions/core
NPOS = N_L0_TRI + N_L1              # 38

TRI_BLOCKS = [sorted({k, 15 - k, 16 + k, 31 - k}) for k in range(NCORES)]


def _tri_positions_for_core(k):
    """(block, jt) list for core k: the 4 diagonal positions first."""
    blocks = TRI_BLOCKS[k]
    diag = [(b, b // 2) for b in blocks]
    off = []
    for b in blocks:
        for jt in range(b // 2 + 1, HW0 // JT):
            off.append((b, jt))
    return diag + off


def _bcast_ap(ap, dim_idx, n):
    """Insert a [0, n] broadcast free-dim at free position dim_idx."""
    new = [list(d) for d in ap.ap]
    new.insert(dim_idx + 1, [0, n])
    return bass.AP(tensor=ap.tensor, offset=ap.offset, ap=new)


def _split_excess_waits(nc, max_waits=1):
    """This walrus build accepts at most one sync-wait per instruction;
    spill extra waits onto preceding same-engine nops."""
    for f in nc.m.functions:
        for b in f.blocks:
            i = 0
            insts = b.instructions
            while i < len(insts):
                inst = insts[i]
                si = inst.sync_info
                if si is not None and si.on_wait and len(si.on_wait) > max_waits:
                    waits = list(si.on_wait)
                    keep = waits[-max_waits:]
                    spill = waits[:-max_waits]
                    si.on_wait = keep
                    inst.sync_info = si
                    new_nops = []
                    for j in range(0, len(spill), max_waits):
                        chunk = spill[j:j + max_waits]
                        nop = nc.engines[inst.engine].nop(nofuse=True).ins
                        nop.sync_info = mybir.SyncInfo(on_wait=chunk, on_update=[])
                        new_nops.append(nop)
                    for f2 in nc.m.functions:
                        for b2 in f2.blocks:
                            for nop in new_nops:
                                if nop in b2.instructions:
                                    b2.instructions.remove(nop)
                    for j, nop in enumerate(new_nops):
                        insts.insert(i + j, nop)
                    i += len(new_nops)
                i += 1


def _build_program():
    nc = bass.Bass()

    def param(name, shape, dt_):
        return nc.declare_dram_parameter(name, shape, dt_, isOutput=False)

    # gathered per-position operand chunks (channel-major: contiguous DMA)
    l0 = [param(f"lg0_{s}", [N_L0_TRI, C0, B, RB], F32) for s in (0, 1)]
    r0 = [param(f"rg0_{s}", [N_L0_TRI, C0, B, JT], F32) for s in (0, 1)]
    l1 = [param(f"l1_{s}", [C1, B, RB], F32) for s in (0, 1)]
    r1 = [param(f"r1_{s}", [C1, B, HW1], F32) for s in (0, 1)]
    masks_in = param("masks", [4, RB, JT], BF16)
    acc_out = nc.declare_dram_parameter("acc", [RB, NPOS], F32, isOutput=True)

    positions = [(0, p, p, p if p < 4 else None) for p in range(N_L0_TRI)]
    positions += [(1, 0, q, None) for q in range(N_L1)]

    with tile.TileContext(nc) as tc:
        with (
            tc.tile_pool(name="resident", bufs=1) as res_pool,
            tc.tile_pool(name="chunks", bufs=3) as chunk_pool,
            tc.tile_pool(name="work", bufs=3) as work_pool,
            tc.tile_pool(name="psum", bufs=2, space="PSUM") as psum_pool,
        ):
            masks = res_pool.tile([RB, 4, JT], BF16, tag="masks")
            nc.sync.dma_start(out=masks, in_=masks_in[:, :, :].rearrange("m p j -> p m j"))

            l1_t, r1_t = [], []
            for s in (0, 1):
                t = res_pool.tile([C1, B, RB], F32, tag=f"l1_{s}")
                nc.sync.dma_start(out=t, in_=l1[s][:, :, :])
                l1_t.append(t)
                t = res_pool.tile([C1, B, HW1], F32, tag=f"r1_{s}")
                nc.sync.dma_start(out=t, in_=r1[s][:, :, :])
                r1_t.append(t)

            acc = res_pool.tile([RB, NPOS], F32, tag="acc")

            for p, (lvl, lsel, rsel, mask_slot) in enumerate(positions):
                if lvl == 0:
                    rch, lch = [], []
                    for s in (0, 1):
                        t = chunk_pool.tile([C0, B, JT], F32, tag=f"rch{s}")
                        nc.sync.dma_start(out=t, in_=r0[s][rsel, :, :, :])
                        rch.append(t)
                        tl = chunk_pool.tile([C0, B, RB], F32, tag=f"lch{s}")
                        nc.sync.dma_start(out=tl, in_=l0[s][lsel, :, :, :])
                        lch.append(tl)

                ps = psum_pool.tile([RB, 2, B, JT], F32, tag="gram")
                for s in (0, 1):
                    for b in range(B):
                        if lvl == 0:
                            lhsT = lch[s][:, b, :]
                            rhs = rch[s][:, b, :]
                        else:
                            lhsT = l1_t[s][:, b, :]
                            rhs = r1_t[s][:, b, rsel * JT:(rsel + 1) * JT]
                        nc.tensor.matmul(ps[:, s, b, :], lhsT, rhs, start=True, stop=True)

                # m = max_b gram (single strided reduce; PSUM has 1 DVE read port)
                mf = work_pool.tile([RB, 2, JT], F32, tag="mf")
                nc.vector.tensor_reduce(
                    out=mf, in_=ps.rearrange("p s b j -> p s j b"),
                    axis=mybir.AxisListType.X, op=mybir.AluOpType.max,
                )
                gsub = work_pool.tile([RB, 2, B, JT], F32, tag="gsub")
                nc.vector.tensor_sub(gsub, ps, _bcast_ap(mf, 1, B))

                e = work_pool.tile([RB, 2, B, JT], BF16, tag="e")
                nc.scalar.activation(out=e, in_=gsub, func=mybir.ActivationFunctionType.Exp)

                spair = work_pool.tile([RB, 2, 2, JT], BF16, tag="spair")
                nc.vector.tensor_add(spair, e[:, :, 0:2, :], e[:, :, 2:4, :])
                ssum = work_pool.tile([RB, 2, JT], F32, tag="ssum")
                nc.vector.tensor_add(ssum, spair[:, :, 0, :], spair[:, :, 1, :])
                rf = work_pool.tile([RB, 2, JT], F32, tag="rinv_f")
                nc.vector.reciprocal(out=rf, in_=ssum)
                rinvb = work_pool.tile([RB, 2, JT], BF16, tag="rinv_b")
                nc.vector.tensor_copy(rinvb, rf)

                a_t = work_pool.tile([RB, 2, B, JT], BF16, tag="a")
                nc.vector.tensor_mul(a_t, e, _bcast_ap(rinvb, 1, B))
                d = work_pool.tile([RB, B, JT], BF16, tag="d")
                nc.vector.tensor_sub(d, a_t[:, 0, :, :], a_t[:, 1, :, :])

                if mask_slot is not None:
                    nc.vector.tensor_mul(d, d, _bcast_ap(masks[:, mask_slot, :], 0, B))

                scratch = work_pool.tile([RB, B, JT], BF16, tag="scratch")
                nc.scalar.activation(
                    out=scratch, in_=d, func=mybir.ActivationFunctionType.Abs,
                    accum_out=acc[:, p:p + 1],
                )

            nc.sync.dma_start(out=acc_out[:, :], in_=acc)

    _split_excess_waits(nc, 1)
    return nc


def _make_in_maps(fea1_0, fea1_1, fea2_0, fea2_1):
    # channel-major (c, b, hw) so every DMA line is contiguous
    r0v = [np.ascontiguousarray(np.asarray(fea1_0, dtype=np.float32).reshape(B, C0, HW0).transpose(1, 0, 2)),
           np.ascontiguousarray(np.asarray(fea2_0, dtype=np.float32).reshape(B, C0, HW0).transpose(1, 0, 2))]
    r1v = [np.ascontiguousarray(np.asarray(fea1_1, dtype=np.float32).reshape(B, C1, HW1).transpose(1, 0, 2)),
           np.ascontiguousarray(np.asarray(fea2_1, dtype=np.float32).reshape(B, C1, HW1).transpose(1, 0, 2))]

    in_maps = []
    for k in range(NCORES):
        m = {}
        row1 = slice(RB * k, RB * (k + 1))
        for s in (0, 1):
            m[f"l1_{s}"] = np.ascontiguousarray(r1v[s][:, :, row1])
            m[f"r1_{s}"] = r1v[s]
        pos = _tri_positions_for_core(k)
        for s in (0, 1):
            lg = np.empty((N_L0_TRI, C0, B, RB), np.float32)
            rg = np.empty((N_L0_TRI, C0, B, JT), np.float32)
            for p, (bl, jt) in enumerate(pos):
                lg[p] = r0v[s][:, :, bl * RB:(bl + 1) * RB]
                rg[p] = r0v[s][:, :, jt * JT:(jt + 1) * JT]
            m[f"lg0_{s}"] = lg
            m[f"rg0_{s}"] = rg
        mask = np.zeros((4, RB, JT), ml_dtypes.bfloat16)
        for slot, (bl, jt) in enumerate(pos[:4]):
            i = np.arange(RB)[:, None]
            j = np.arange(JT)[None, :]
            gi = bl * RB + i
            gj = jt * JT + j
            w = np.where(gj > gi, 1.0, np.where(gj == gi, 0.5, 0.0))
            mask[slot] = w.astype(ml_dtypes.bfloat16)
        m["masks"] = mask
        in_maps.append(m)
    return in_maps


_NC_CACHE = None


def kernel(fea1_0, fea1_1, fea2_0, fea2_1):
    global _NC_CACHE
    if _NC_CACHE is None:
        _NC_CACHE = _build_program()
    nc = _NC_CACHE
    in_maps = _make_in_maps(fea1_0, fea1_1, fea2_0, fea2_1)
    res = run_bass_kernel_spmd(nc, in_maps, core_ids=list(range(NCORES)))
    s0 = 0.0
    s1 = 0.0
    for r in res.results:
        acc = r["acc"].astype(np.float64)
        s0 += acc[:, :N_L0_TRI].sum()
        s1 += acc[:, N_L0_TRI:].sum()
    s0 *= 2.0  # upper triangle with {1, .5, 0} diag masks -> full-matrix sum
    loss = 0.5 * (s0 / (B * HW0 * HW0) + s1 / (B * HW1 * HW1))
    return np.float32(loss)
